# revision 3
# baseline (speedup 1.0000x reference)
"""Trainium2 Bass kernel for a pre-LN transformer block (B=256, T=200, E=384).

Data-parallel over batch: 8 NeuronCores x 32 batches. v2: software-pipelined
octet loop (LN/transposes of octet o+1 overlap FFN of octet o), batched xbar
transposes (1 per token tile instead of 3), SWDGE (gpsimd) bulk loads/stores,
causal masks on gpsimd, in-place residual stream, K=1 bf16 broadcast matmuls,
single fused attention normalize.

Layouts:
  - Residual stream token-major [128, 13, 384] f32, updated in place.
  - Activations feature-major via 3D-out dma_start_transpose:
    hT[a, k, 128*i+c] = h[c, i, 128*k+a].
  - Attention: scoresT (keys on partitions); exp unmasked (|scores| small),
    mask multiply on gpsimd; denominators via ones-matmul; recip broadcast to
    128 partitions via K=1 bf16 matmuls; one tensor_tensor normalize per batch
    reading two PSUM operands.
"""

import numpy as np
import ml_dtypes

B, T, E, F, NH, HS = 256, 200, 384, 1536, 6, 64
NCORES = 8
BPC = B // NCORES          # batches per core = 32
G = 8                      # batches per octet
NOCT = BPC // G            # 4
TOK = G * T                # 1600 tokens per octet
NT = 13                    # token tiles per octet: 12x128 + 1x64
TW = [128] * 12 + [64]     # tile widths
NCH = 4                    # 400-wide column chunks of TOK
CH = TOK // NCH            # 400

_CACHE = {}

# tunables
USE_SWDGE = False          # bulk loads/stores via gpsimd (SWDGE)
MASK_ENGINE = "gpsimd"     # causal mask multiplies: "gpsimd" or "vector"
FFN1_DVE_EVERY = 4         # every Nth FFN1 relu-copy goes to DVE (0 = none)
V_COPY_ENGINE = "vector"   # v PSUM->SBUF copies
TRANSPOSE_3D = True        # batched per-tile transpose (3D out AP)
USE_STAGE = True           # stage odd-head q/k rows down to partition 0
                           # (matmuls with base_partition=64 operands crash at
                           # runtime on this walrus)
TR_RINGS = 1               # transposes on both HWDGE rings corrupt data —
                           # keep them all on the SP ring


def _install_drain_patch():
    """walrus in this container allows only one sem wait on a Drain; split the
    TileContext exit drain into a chain of single-wait drains."""
    import concourse.tile as tile
    import bass_rust
    from concourse.vector_clock import ScopedClock

    if getattr(tile.TileContext, "_drain_patch", False):
        return

    def _patched(self, tick_clock, wait_clock):
        nc = self.nc
        drain_inst = nc.sync.drain()
        wait_clock.add_sem_waits(
            drain_inst.ins, ScopedClock({None: tick_clock.global_clock})
        )
        si = drain_inst.ins.sync_info
        waits = list(si.on_wait) if si is not None else []
        if len(waits) > 1:
            si.on_wait = waits[:1]
            drain_inst.ins.sync_info = si
            for w in waits[1:]:
                d2 = nc.sync.drain()
                d2.ins.sync_info = bass_rust.SyncInfo(on_wait=[w], on_update=[])
        nc.all_engine_barrier()
        assert self.sems is not None
        popped = nc._tile_sem_poison_stack.pop()
        assert popped is self._sem_poison
        nc.clear_and_free_semaphores(list(self.sems.allocated().values()))
        nc.all_engine_barrier()

    tile.TileContext._drain_and_barrier = _patched
    tile.TileContext._drain_patch = True


def _install_wait_split_patch():
    """walrus here supports only one sync-wait per instruction on several
    templates. Split any multi-wait instruction at the BIR-JSON level into a
    chain of single-wait Drain instructions on the same engine, inserted
    immediately before it."""
    import json
    import concourse.bass_utils as bu
    import concourse.bass2jax as b2j

    if getattr(bu, "_wait_split_patch", False):
        return
    orig = bu.compile_bir_kernel

    def patched(bir_json, tmpdir, neff_name="file.neff"):
        d = json.loads(bir_json)
        uid = [0]
        ndrop = [0]
        for fn in d.get("functions", []):
            for bb in fn.get("blocks", []):
                new_insts = []
                # Drop PE Ldweights that reload the exact weights already in
                # the array (same AP as the previous Ldweights in this block's
                # PE stream, no syncs attached, no weight-path use between).
                last_ldw = [None]
                for ins in bb.get("instructions", []):
                    si = ins.get("sync_info") or {}
                    waits = si.get("on_wait") or []
                    if ins.get("engine") == "PE":
                        op = ins.get("opcode")
                        if op == "Ldweights":
                            key = json.dumps(
                                [ins.get("ins"), ins.get("tile_position"),
                                 ins.get("tile_size"), ins.get("perf_mode"),
                                 ins.get("is_transpose")],
                                sort_keys=True)
                            if (key == last_ldw[0] and not waits
                                    and not (si.get("on_update") or [])):
                                ndrop[0] += 1
                                continue
                            last_ldw[0] = key
                        elif op == "Matmult":
                            if ins.get("ldweights"):
                                last_ldw[0] = None
                        elif op not in ("Drain", "EventSemaphore"):
                            last_ldw[0] = None
                    if len(waits) > 1:
                        for w in waits[:-1]:
                            uid[0] += 1
                            new_insts.append({
                                "debug": ins.get("debug", 0),
                                "engine": ins["engine"],
                                "ins": [],
                                "outs": [],
                                "is_reset_sema": False,
                                "name": f"WSPLIT-{uid[0]}",
                                "opcode": "Drain",
                                "sync_info": {"on_update": [],
                                              "on_wait": [w]},
                            })
                        si["on_wait"] = [waits[-1]]
                        ins["sync_info"] = si
                    new_insts.append(ins)
                bb["instructions"] = new_insts
        if ndrop[0]:
            print(f"[kernel2] deduped {ndrop[0]} redundant Ldweights")
        return orig(json.dumps(d).encode(), tmpdir, neff_name=neff_name)

    bu.compile_bir_kernel = patched
    b2j.compile_bir_kernel = patched
    bu._wait_split_patch = True


def _build_nc(n_octets=NOCT, stage=99, loop_reps=None):
    import concourse.bass as bass
    import concourse.mybir as mybir
    import concourse.tile as tile

    _install_drain_patch()
    f32 = mybir.dt.float32
    bf16 = mybir.dt.bfloat16
    AF = mybir.ActivationFunctionType
    OP = mybir.AluOpType

    nc = bass.Bass("TRN2")

    x_d = nc.dram_tensor("x", [BPC, T, E], bf16, kind="ExternalInput")
    wq_d = nc.dram_tensor("wq", [E, E], bf16, kind="ExternalInput")
    wk_d = nc.dram_tensor("wk", [E, E], bf16, kind="ExternalInput")
    wv_d = nc.dram_tensor("wv", [E, E], bf16, kind="ExternalInput")
    wp_d = nc.dram_tensor("wp", [E, E], bf16, kind="ExternalInput")
    w1_d = nc.dram_tensor("w1", [E, F], bf16, kind="ExternalInput")
    w2_d = nc.dram_tensor("w2", [F, E], bf16, kind="ExternalInput")
    cq_d = nc.dram_tensor("cq", [E], f32, kind="ExternalInput")
    ck_d = nc.dram_tensor("ck", [E], f32, kind="ExternalInput")
    b1_d = nc.dram_tensor("b1p", [F], f32, kind="ExternalInput")
    bp_d = nc.dram_tensor("bpb", [1, E], bf16, kind="ExternalInput")
    b2_d = nc.dram_tensor("b2b", [1, E], bf16, kind="ExternalInput")
    m0_d = nc.dram_tensor("m0", [128, NH, 128], bf16, kind="ExternalInput")
    m1_d = nc.dram_tensor("m1", [72, NH, 72], bf16, kind="ExternalInput")
    oc_d = nc.dram_tensor("onc", [128, 1], bf16, kind="ExternalInput")
    or_d = nc.dram_tensor("onr", [1, 128], bf16, kind="ExternalInput")
    it0_d = nc.dram_tensor("ind0", [1, 128], bf16, kind="ExternalInput")
    it1_d = nc.dram_tensor("ind1", [1, 128], bf16, kind="ExternalInput")
    y_d = nc.dram_tensor("y", [BPC, T, E], f32, kind="ExternalOutput")

    x_flat = x_d[:].rearrange("b t d -> (b t) d")
    y_flat = y_d[:].rearrange("b t d -> (b t) d")

    ld_eng = nc.gpsimd if USE_SWDGE else nc.sync
    mask_eng = nc.gpsimd if MASK_ENGINE == "gpsimd" else nc.vector
    vcp_eng = nc.vector if V_COPY_ENGINE == "vector" else nc.scalar

    from contextlib import ExitStack

    with tile.TileContext(nc) as tc, ExitStack() as es:
        cpool = es.enter_context(tc.tile_pool(name="const", bufs=1))
        upool = es.enter_context(tc.tile_pool(name="uc", bufs=2))
        ypool = es.enter_context(tc.tile_pool(name="yc", bufs=2))
        xpool = es.enter_context(tc.tile_pool(name="xres", bufs=2))
        hpool = es.enter_context(tc.tile_pool(name="hb", bufs=2))
        htpool = es.enter_context(tc.tile_pool(name="htb", bufs=2))
        stpool = es.enter_context(tc.tile_pool(name="st", bufs=2))
        spool = es.enter_context(tc.tile_pool(name="big", bufs=1))
        epool = es.enter_context(tc.tile_pool(name="exp", bufs=2))
        opool = (es.enter_context(tc.tile_pool(name="out", bufs=3))
                 if stage < 99 else None)
        ppool = es.enter_context(tc.tile_pool(name="ps", bufs=3, space="PSUM"))
        papool = es.enter_context(tc.tile_pool(name="pa", bufs=2, space="PSUM"))
        p1pool = es.enter_context(tc.tile_pool(name="ps1", bufs=1, space="PSUM"))

        # ---- constants ----
        wq_s = cpool.tile([128, 3, E], bf16, tag="wq")
        wk_s = cpool.tile([128, 3, E], bf16, tag="wk")
        wv_s = cpool.tile([128, 3, E], bf16, tag="wv")
        wp_s = cpool.tile([128, 3, E], bf16, tag="wp")
        w1_s = cpool.tile([128, 3, F], bf16, tag="w1")
        w2_s = cpool.tile([128, 12, E], bf16, tag="w2")
        for dst, src in ((wq_s, wq_d), (wk_s, wk_d), (wv_s, wv_d), (wp_s, wp_d),
                         (w1_s, w1_d), (w2_s, w2_d)):
            nc.sync.dma_start(dst[:], src[:].rearrange("(ko p) m -> p ko m", p=128))
        cq_s = cpool.tile([128, 3], f32, tag="cq")
        ck_s = cpool.tile([128, 3], f32, tag="ck")
        b1_s = cpool.tile([128, 12], f32, tag="b1")
        nc.sync.dma_start(cq_s[:], cq_d[:].rearrange("(mo p) -> p mo", p=128))
        nc.sync.dma_start(ck_s[:], ck_d[:].rearrange("(mo p) -> p mo", p=128))
        nc.sync.dma_start(b1_s[:], b1_d[:].rearrange("(mo p) -> p mo", p=128))
        bp_s = cpool.tile([1, E], bf16, tag="bp")
        b2_s = cpool.tile([1, E], bf16, tag="b2")
        nc.sync.dma_start(bp_s[:], bp_d[:])
        nc.sync.dma_start(b2_s[:], b2_d[:])
        m0_s = cpool.tile([128, NH, 128], bf16, tag="m0")
        m1_s = cpool.tile([72, NH, 72], bf16, tag="m1")
        oc_s = cpool.tile([128, 1], bf16, tag="onc")
        or_s = cpool.tile([1, 128], bf16, tag="onr")
        it0_s = cpool.tile([1, 128], bf16, tag="ind0")
        it1_s = cpool.tile([1, 128], bf16, tag="ind1")
        eps_s = cpool.tile([128, 1], f32, tag="eps")
        nc.vector.memset(eps_s[:], 1e-5)
        id_d = nc.dram_tensor("idm", [128, 128], bf16, kind="ExternalInput")
        id_s = cpool.tile([128, 128], bf16, tag="idm")
        nc.sync.dma_start(id_s[:], id_d[:])
        nc.sync.dma_start(m0_s[:], m0_d[:])
        nc.sync.dma_start(m1_s[:], m1_d[:])
        nc.sync.dma_start(oc_s[:], oc_d[:])
        nc.sync.dma_start(or_s[:], or_d[:])
        nc.sync.dma_start(it0_s[:], it0_d[:])
        nc.sync.dma_start(it1_s[:], it1_d[:])

        # transpose-ring round-robin between the two HWDGE rings (SP / ACT)
        tr_state = [0]

        def transpose_tile(hT, h_tile, i, w):
            eng = nc.sync if (TR_RINGS == 1 or tr_state[0] % 2 == 0) else nc.scalar
            tr_state[0] += 1
            if TRANSPOSE_3D:
                eng.dma_start_transpose(
                    hT[:, 0:3, 128 * i : 128 * i + w], h_tile[:w, i, :]
                )
            else:
                for k in range(3):
                    eng.dma_start_transpose(
                        hT[:, k, 128 * i : 128 * i + w],
                        h_tile[:w, i, 128 * k : 128 * (k + 1)],
                    )

        def load_x(o):
            x_t = xpool.tile([128, NT, E], bf16, tag="x", name=f"x{o}")
            r0 = o * TOK
            ld_eng.dma_start(
                x_t[:, 0:12, :],
                x_flat[r0 : r0 + 1536].rearrange("(g p) d -> p g d", p=128),
            )
            ld_eng.dma_start(x_t[0:64, 12, :], x_flat[r0 + 1536 : r0 + 1600])
            return x_t

        def store_chunk(y_c, o, c):
            # store the 4 (or 1) token tiles of ffn chunk c
            r0 = o * TOK + 512 * c
            if c < 3:
                ld_eng.dma_start(
                    y_flat[r0 : r0 + 512].rearrange("(g p) d -> p g d", p=128),
                    y_c[:, 0:4, :],
                )
            else:
                ld_eng.dma_start(y_flat[r0 : r0 + 64], y_c[0:64, 0, :])

        def ln_stats_tile(stats, x_t, i):
            w = TW[i]
            nc.vector.bn_stats(stats[:w, i, :], x_t[:w, i, :])

        def ln_alloc(name):
            mv = stpool.tile([128, NT, 2], f32, tag="mv", name=f"mv{name}")
            nc.vector.memset(mv[:], 1.0)
            sd = stpool.tile([128, NT], f32, tag="sd", name=f"sd{name}")
            av = stpool.tile([128, NT], f32, tag="av", name=f"av{name}")
            b0 = stpool.tile([128, NT], f32, tag="b0", name=f"b0{name}")
            h_t = hpool.tile([128, NT, E], bf16, tag="h", name=f"h{name}")
            hT = htpool.tile([128, 3, TOK], bf16, tag="hT", name=f"hT{name}")
            return (mv, sd, av, b0, h_t, hT)

        def ln_group(ln, stats, x_t, lo, hi):
            """sd/av/b0 + normalize + transpose for tiles [lo, hi). bn_aggr
            for these tiles must already have run."""
            mv, sd, av, b0, h_t, hT = ln
            nc.scalar.activation(
                sd[:, lo:hi], mv[:, lo:hi, 1], AF.Sqrt, bias=eps_s[:, 0:1]
            )
            nc.vector.reciprocal(av[:, lo:hi], sd[:, lo:hi])
            nc.vector.tensor_tensor(
                b0[:, lo:hi], mv[:, lo:hi, 0], av[:, lo:hi], OP.mult
            )
            nc.vector.tensor_scalar(b0[:, lo:hi], b0[:, lo:hi], -1.0, None, OP.mult)
            for i in range(lo, hi):
                w = TW[i]
                nc.vector.tensor_scalar(
                    h_t[:w, i, :], x_t[:w, i, :],
                    av[:w, i : i + 1], b0[:w, i : i + 1], OP.mult, OP.add,
                )
                transpose_tile(hT, h_t, i, w)

        def ln_finish(stats, x_t, name):
            """aggr + normalize into a new h tile + transposes into a new hT."""
            ln = ln_alloc(name)
            mv = ln[0]
            for i in range(NT):
                w = TW[i]
                nc.vector.bn_aggr(mv[:w, i, :], stats[:w, i, :])
            ln_group(ln, stats, x_t, 0, NT)
            return ln[5]

        def qkv(hT, name):
            """qT/kT feature-major + v token-major for one octet. Chunk-pair
            inner loops keep the stationary weights loaded across 2 matmuls."""
            qT = spool.tile([128, 3, TOK], bf16, tag="qT", name=f"qT{name}")
            kT = spool.tile([128, 3, TOK], bf16, tag="kT", name=f"kT{name}")
            for dstT, w_s, c_s in ((qT, wq_s, cq_s), (kT, wk_s, ck_s)):
                for m in range(3):
                    for cp in (0, 2):
                        pq = {}
                        for c in (cp, cp + 1):
                            pq[c] = ppool.tile([128, 512], f32, tag="pb", name="pq")
                        for k in range(3):
                            for c in (cp, cp + 1):
                                nc.tensor.matmul(
                                    pq[c][:, 0:CH],
                                    w_s[:, k, 128 * m : 128 * (m + 1)],
                                    hT[:, k, CH * c : CH * (c + 1)],
                                    start=(k == 0), stop=(k == 2),
                                )
                        for c in (cp, cp + 1):
                            nc.scalar.activation(
                                dstT[:, m, CH * c : CH * (c + 1)],
                                pq[c][:, 0:CH],
                                AF.Identity, bias=c_s[:, m : m + 1],
                            )
            v_all = spool.tile([128, G, 2, E], bf16, tag="v", name=f"v{name}")
            for b in range(G):
                for tt in range(2):
                    w = 128 if tt == 0 else 72
                    col = 200 * b + 128 * tt
                    pv = ppool.tile([128, 512], f32, tag="pb")
                    for k in range(3):
                        nc.tensor.matmul(
                            pv[:w, 0:E],
                            hT[:, k, col : col + w],
                            wv_s[:, k, :],
                            start=(k == 0), stop=(k == 2),
                        )
                    vcp_eng.tensor_copy(v_all[:w, b, tt, :], pv[:w, 0:E])
            # bulk-stage the odd heads' rows down to partition base 0 (matmul
            # operands at base_partition 64 crash this walrus at runtime)
            qstg = spool.tile([64, 3, TOK], bf16, tag="qstg", name=f"qs{name}")
            kstg = spool.tile([64, 3, TOK], bf16, tag="kstg", name=f"ks{name}")
            nc.sync.dma_start(qstg[:], qT[64:128, :, :])
            nc.sync.dma_start(kstg[:], kT[64:128, :, :])
            return qT, kT, v_all, qstg, kstg

        def dump(tile_ap, nrows, row0, ncols=E):
            d = opool.tile([128, E], f32, tag="ot")
            nc.vector.tensor_copy(d[:nrows, :ncols], tile_ap)
            nc.sync.dma_start(y_flat[row0 : row0 + nrows], d[:nrows, :])

        # ================= main body =================
        loop_cm = None
        if loop_reps is not None:
            loop_cm = tc.For_i(0, loop_reps, 1)
            loop_cm.__enter__()

        # prologue (inside the loop body so each rep is self-contained)
        x_t = {}
        x_t[0] = load_x(0)
        stats0 = stpool.tile([128, NT, 6], f32, tag="stats", name="statsP")
        for i in range(NT):
            ln_stats_tile(stats0, x_t[0], i)
        hT0 = ln_finish(stats0, x_t[0], "P")
        qkv_cur = qkv(hT0, "P")

        for o in range(n_octets):
            nxt = (o + 1) % NOCT
            prep_nxt = o < n_octets - 1
            r0 = o * TOK
            qT, kT, v_all, qstg_o, kstg_o = qkv_cur

            # ---- load x(nxt) early ----
            if prep_nxt:
                x_t[nxt] = load_x(nxt)

            if stage <= 2:
                # h/hT debug: recompute dump from qT
                for i in range(4):
                    dump(qT[:, 0, 384 * i : 384 * (i + 1)], 128, r0 + 128 * i)
                qkv_cur = qkv_cur  # keep
                continue

            # ---- attention (PE runs scores one batch ahead of softmax),
            # with proj tiles interleaved as their attT columns complete ----
            attT = spool.tile([128, 3, TOK], bf16, tag="attT", name=f"attT{o}")
            stats2 = stpool.tile([128, NT, 6], f32, tag="stats", name=f"st2_{o}")
            ln2 = ln_alloc(f"B{o}")
            GRPS = [4, 8, NT]
            ln_state = [0, 0]  # g_lo, proj_done

            def proj_tile(i):
                w = TW[i]
                pp = ppool.tile([128, 512], f32, tag="pb", name="pp")
                for k in range(3):
                    nc.tensor.matmul(
                        pp[:w, 0:E],
                        attT[:, k, 128 * i : 128 * i + w],
                        wp_s[:, k, :],
                        start=(k == 0), stop=False,
                    )
                nc.tensor.matmul(
                    pp[:w, 0:E], or_s[0:1, 0:w], bp_s[:],
                    start=False, stop=False,
                )
                nc.tensor.matmul(
                    pp[:w, 0:E], id_s[:w, :w], x_t[o][:w, i, :],
                    start=False, stop=True,
                )
                nc.scalar.activation(x_t[o][:w, i, :], pp[:w, 0:E], AF.Copy)
                ln_stats_tile(stats2, x_t[o], i)
                nc.vector.bn_aggr(ln2[0][:w, i, :], stats2[:w, i, :])
                if i + 1 in GRPS:
                    ln_group(ln2, stats2, x_t[o], ln_state[0], i + 1)
                    ln_state[0] = i + 1

            pend = None
            for b in range(G + 1):
                cur = None
                if b < G:
                    c0 = 200 * b
                    expT0 = epool.tile([128, NH, T], bf16, tag="expT0")
                    expT1 = epool.tile([72, NH, 72], bf16, tag="expT1")
                    rcp = epool.tile([1, NH, T], bf16, tag="rcp")
                    def kslice(j, r, lo, hi):
                        if r == 0 or not USE_STAGE:
                            return kT[64 * r : 64 * r + 64, j, c0 + lo : c0 + hi]
                        return kstg_o[:, j, c0 + lo : c0 + hi]

                    def qslice(j, r, lo, hi):
                        if r == 0 or not USE_STAGE:
                            return qT[64 * r : 64 * r + 64, j, c0 + lo : c0 + hi]
                        return qstg_o[:, j, c0 + lo : c0 + hi]

                    # scores part1: keys 0-127, all queries
                    ps_list = []
                    for j in range(3):
                        ps = ppool.tile([128, 2, 256], f32, tag="pb", name="ps")
                        ps = ps[:, :, 0:T]
                        ps_list.append(ps)
                        for r in range(2):
                            nc.tensor.matmul(
                                ps[:, r, :],
                                kslice(j, r, 0, 128),
                                qslice(j, r, 0, T),
                                start=True, stop=True,
                            )
                    # scores part2: keys 128-200, queries 128-200
                    ps1 = p1pool.tile([72, NH, 72], f32, tag="ps1")
                    for j in range(3):
                        for r in range(2):
                            h = 2 * j + r
                            nc.tensor.matmul(
                                ps1[:, h, :],
                                kslice(j, r, 128, 200),
                                qslice(j, r, 128, 200),
                                start=True, stop=True,
                            )
                    for j in range(3):
                        nc.scalar.activation(
                            expT0[:, 2 * j : 2 * j + 2, :], ps_list[j][:], AF.Exp
                        )
                    nc.scalar.activation(expT1[:], ps1[:], AF.Exp)
                    for j in range(3):
                        mask_eng.tensor_tensor(
                            expT0[:, 2 * j : 2 * j + 2, 0:128],
                            expT0[:, 2 * j : 2 * j + 2, 0:128],
                            m0_s[:, 2 * j : 2 * j + 2, :], OP.mult,
                        )
                    # expT1's mask on DVE: it gates every denominator tail
                    # matmul, so don't queue it behind the gpsimd masks
                    nc.vector.tensor_tensor(expT1[:], expT1[:], m1_s[:],
                                            OP.mult)
                    cur = (expT0, expT1, rcp, c0)

                if pend is not None:
                    expT0, expT1, rcp, c0 = pend
                    bb = b - 1
                    sm_list = []
                    for j in range(3):
                        sm = ppool.tile([1, 512], f32, tag="pb", name="sm")
                        sm = sm[:, 0 : 2 * T].rearrange("p (h t) -> p h t", h=2)
                        sm_list.append(sm)
                        nc.tensor.matmul(
                            sm[:], oc_s[:, :], expT0[:, 2 * j : 2 * j + 2, :],
                            start=True, stop=False,
                        )
                        for r in range(2):
                            nc.tensor.matmul(
                                sm[:, r, 128:200], oc_s[0:72, :],
                                expT1[:, 2 * j + r, :],
                                start=False, stop=(r == 1),
                            )
                    with nc.allow_low_precision(reason="softmax recip in bf16"):
                        for j in range(3):
                            nc.vector.reciprocal(
                                rcp[0:1, 2 * j : 2 * j + 2, :], sm_list[j][:]
                            )

                    # attV (unnormalized) into pa banks
                    pa = papool.tile([128, 3, 256], f32, tag="pab")
                    for j in range(3):
                        for r in range(2):
                            h = 2 * j + r
                            nc.tensor.matmul(
                                pa[64 * r : 64 * r + 64, j, 0:T],
                                v_all[0:128, bb, 0, 64 * h : 64 * h + 64],
                                expT0[:, h, :],
                                start=True, stop=False,
                            )
                            nc.tensor.matmul(
                                pa[64 * r : 64 * r + 64, j, 128:200],
                                v_all[0:72, bb, 1, 64 * h : 64 * h + 64],
                                expT1[:, h, :],
                                start=False, stop=True,
                            )

                    # broadcast recip across partitions (K=1 bf16 matmuls)
                    rb = papool.tile([128, 3, 256], f32, tag="pab")
                    for j in range(3):
                        nc.tensor.matmul(
                            rb[:, j, 0:T], it0_s[:], rcp[0:1, 2 * j, :],
                            start=True, stop=False,
                        )
                        nc.tensor.matmul(
                            rb[:, j, 0:T], it1_s[:], rcp[0:1, 2 * j + 1, :],
                            start=False, stop=True,
                        )
                    rbs = epool.tile([128, 3, T], bf16, tag="rbs")
                    nc.scalar.activation(rbs[:], rb[:, :, 0:T], AF.Copy)
                    # normalize: attT[:, :, c0:c0+T] = pa * rbs  (one DVE op)
                    nc.vector.tensor_tensor(
                        attT[:, 0:3, c0 : c0 + T], pa[:, :, 0:T], rbs[:],
                        OP.mult,
                    )
                pend = cur

            if stage <= 5:
                for i in range(4):
                    dump(attT[:, 0, 384 * i : 384 * (i + 1)], 128, r0 + 128 * i)
                qkv_cur = qkv(ln_finish(stats0, x_t[nxt], f"S{o}"), f"S{o}")
                continue

            # ---- proj tail: tiles not yet emitted in the attention loop ----
            for i in range(ln_state[1], NT):
                proj_tile(i)
            h2T = ln2[5]

            if stage <= 6:
                for i in range(NT):
                    w = TW[i]
                    dump(x_t[o][:w, i, :], w, r0 + 128 * i)
                qkv_cur = qkv(ln_finish(stats0, x_t[nxt], f"S{o}"), f"S{o}")
                continue

            # ---- fused FFN: 512-token chunks, FFN1 -> FFN2 -> store ----
            # chunk token ranges: [0,512) [512,1024) [1024,1536) [1536,1600)
            ffn1_idx = [0]
            CWS = [512, 512, 512, 64]

            def ffn1_chunks(cs):
                """FFN1 for a group of chunks with shared stationary w1."""
                ucs = {}
                for c in cs:
                    ucs[c] = upool.tile([128, 12, 512], bf16, tag="uc", name="uc")
                for m in range(12):
                    pu = {}
                    for c in cs:
                        pu[c] = ppool.tile([128, 512], f32, tag="pb", name="pu")
                    for k in range(3):
                        for c in cs:
                            nc.tensor.matmul(
                                pu[c][:, 0 : CWS[c]],
                                w1_s[:, k, 128 * m : 128 * (m + 1)],
                                h2T[:, k, 512 * c : 512 * c + CWS[c]],
                                start=(k == 0), stop=(k == 2),
                            )
                    for c in cs:
                        ffn1_idx[0] += 1
                        if FFN1_DVE_EVERY and ffn1_idx[0] % FFN1_DVE_EVERY == 0:
                            nc.vector.tensor_scalar(
                                ucs[c][:, m, 0 : CWS[c]], pu[c][:, 0 : CWS[c]],
                                b1_s[:, m : m + 1], 0.0, OP.add, OP.max,
                            )
                        else:
                            nc.scalar.activation(
                                ucs[c][:, m, 0 : CWS[c]], pu[c][:, 0 : CWS[c]],
                                AF.Relu, bias=b1_s[:, m : m + 1],
                            )
                return ucs

            def ffn2_chunk(uc, c):
                ntile = 4 if c < 3 else 1
                y_c = ypool.tile([128, 4, E], f32, tag="yc")
                for t in range(ntile):
                    i = 4 * c + t
                    w = TW[i]
                    pf = ppool.tile([128, 512], f32, tag="pb", name="pf")
                    for k in range(12):
                        nc.tensor.matmul(
                            pf[:w, 0:E],
                            uc[:, k, 128 * t : 128 * t + w],
                            w2_s[:, k, :],
                            start=(k == 0), stop=False,
                        )
                    nc.tensor.matmul(
                        pf[:w, 0:E], or_s[0:1, 0:w], b2_s[:],
                        start=False, stop=False,
                    )
                    nc.tensor.matmul(
                        pf[:w, 0:E], id_s[:w, :w], x_t[o][:w, i, :],
                        start=False, stop=True,
                    )
                    nc.scalar.activation(y_c[:w, t, :], pf[:w, 0:E], AF.Copy)
                store_chunk(y_c, o, c)

            ucs = ffn1_chunks([0])
            ffn2_chunk(ucs[0], 0)
            ucs2 = ffn1_chunks([1, 2])
            ffn2_chunk(ucs2[1], 1)

            # ---- LN1(nxt): stats + finish + transposes ----
            hT_nxt = None
            if prep_nxt:
                stats1 = stpool.tile([128, NT, 6], f32, tag="stats",
                                     name=f"st1_{o}")
                for i in range(NT):
                    ln_stats_tile(stats1, x_t[nxt], i)
                hT_nxt = ln_finish(stats1, x_t[nxt], f"N{o}")

            ffn2_chunk(ucs2[2], 2)
            ucs3 = ffn1_chunks([3])
            ffn2_chunk(ucs3[3], 3)

            # ---- QKV(nxt) ----
            if prep_nxt:
                qkv_cur = qkv(hT_nxt, f"Q{o}")

        if loop_cm is not None:
            loop_cm.__exit__(None, None, None)

    return nc


def _prep_inputs(inputs):
    """Host-side folding of LN gains/biases into weights. Exact in fp32."""
    bf = ml_dtypes.bfloat16
    x = np.asarray(inputs["x"], np.float32)
    Wq = np.asarray(inputs["Wq"], np.float32)
    Wk = np.asarray(inputs["Wk"], np.float32)
    Wv = np.asarray(inputs["Wv"], np.float32)
    Wp = np.asarray(inputs["Wproj"], np.float32)
    bproj = np.asarray(inputs["bproj"], np.float32)
    W1 = np.asarray(inputs["W1"], np.float32)
    b1 = np.asarray(inputs["b1"], np.float32)
    W2 = np.asarray(inputs["W2"], np.float32)
    b2 = np.asarray(inputs["b2"], np.float32)
    g1 = np.asarray(inputs["g1"], np.float32)
    be1 = np.asarray(inputs["be1"], np.float32)
    g2 = np.asarray(inputs["g2"], np.float32)
    be2 = np.asarray(inputs["be2"], np.float32)

    s = E ** -0.5
    wq_f = (g1[:, None] * Wq) * s
    wk_f = g1[:, None] * Wk
    wv_f = g1[:, None] * Wv
    cq = (be1 @ Wq) * s
    ck = be1 @ Wk
    cv = be1 @ Wv
    bp_f = bproj + cv @ Wp
    w1_f = g2[:, None] * W1
    b1_f = b1 + be2 @ W1

    m0 = np.zeros((128, NH, 128), np.float32)
    sidx = np.arange(128)[:, None]
    tidx = np.arange(128)[None, :]
    m0[:, :, :] = (tidx >= sidx)[:, None, :]
    m1 = np.zeros((72, NH, 72), np.float32)
    si = np.arange(72)[:, None]
    ti = np.arange(72)[None, :]
    m1[:, :, :] = (ti >= si)[:, None, :]

    ind0 = np.zeros((1, 128), np.float32); ind0[0, 0:64] = 1.0
    ind1 = np.zeros((1, 128), np.float32); ind1[0, 64:128] = 1.0

    common = {
        "wq": wq_f.astype(bf), "wk": wk_f.astype(bf), "wv": wv_f.astype(bf),
        "wp": Wp.astype(bf), "w1": w1_f.astype(bf), "w2": W2.astype(bf),
        "cq": cq, "ck": ck, "b1p": b1_f,
        "bpb": bp_f.astype(bf).reshape(1, E), "b2b": b2.astype(bf).reshape(1, E),
        "m0": m0.astype(bf), "m1": m1.astype(bf),
        "onc": np.ones((128, 1), bf), "onr": np.ones((1, 128), bf),
        "ind0": ind0.astype(bf), "ind1": ind1.astype(bf),
        "idm": np.eye(128, dtype=np.float32).astype(bf),
    }
    return x.astype(bf), common


def kernel(**inputs):
    from concourse.bass_utils import run_bass_kernel_spmd

    _install_wait_split_patch()

    x, common = _prep_inputs(inputs)
    if "nc" not in _CACHE:
        _CACHE["nc"] = _build_nc()
    nc = _CACHE["nc"]
    in_maps = []
    for c in range(NCORES):
        m = dict(common)
        m["x"] = np.ascontiguousarray(x[c * BPC : (c + 1) * BPC])
        in_maps.append(m)
    res = run_bass_kernel_spmd(nc, in_maps, core_ids=list(range(NCORES)))
    out = np.concatenate([res.results[c]["y"] for c in range(NCORES)], axis=0)
    return out.astype(np.float32)


# revision 4
# speedup vs baseline: 1.0454x; 1.0454x over previous
"""Trainium2 Bass kernel for a pre-LN transformer block (B=256, T=200, E=384).

Data-parallel over batch: 8 NeuronCores x 32 batches. v2: software-pipelined
octet loop (LN/transposes of octet o+1 overlap FFN of octet o), batched xbar
transposes (1 per token tile instead of 3), SWDGE (gpsimd) bulk loads/stores,
causal masks on gpsimd, in-place residual stream, K=1 bf16 broadcast matmuls,
single fused attention normalize.

Layouts:
  - Residual stream token-major [128, 13, 384] f32, updated in place.
  - Activations feature-major via 3D-out dma_start_transpose:
    hT[a, k, 128*i+c] = h[c, i, 128*k+a].
  - Attention: scoresT (keys on partitions); exp unmasked (|scores| small),
    mask multiply on gpsimd; denominators via ones-matmul; recip broadcast to
    128 partitions via K=1 bf16 matmuls; one tensor_tensor normalize per batch
    reading two PSUM operands.
"""

import numpy as np
import ml_dtypes

B, T, E, F, NH, HS = 256, 200, 384, 1536, 6, 64
NCORES = 8
BPC = B // NCORES          # batches per core = 32
G = 8                      # batches per octet
NOCT = BPC // G            # 4
TOK = G * T                # 1600 tokens per octet
NT = 13                    # token tiles per octet: 12x128 + 1x64
TW = [128] * 12 + [64]     # tile widths
NCH = 4                    # 400-wide column chunks of TOK
CH = TOK // NCH            # 400

_CACHE = {}

# tunables
USE_SWDGE = False          # bulk loads/stores via gpsimd (SWDGE)
MASK_ENGINE = "gpsimd"     # causal mask multiplies: "gpsimd" or "vector"
FFN1_DVE_EVERY = 4         # every Nth FFN1 relu-copy goes to DVE (0 = none)
V_COPY_ENGINE = "vector"   # v PSUM->SBUF copies
TRANSPOSE_3D = True        # batched per-tile transpose (3D out AP)
USE_STAGE = True           # stage odd-head q/k rows down to partition 0
                           # (matmuls with base_partition=64 operands crash at
                           # runtime on this walrus)
TR_RINGS = 1               # transposes on both HWDGE rings corrupt data —
                           # keep them all on the SP ring


def _install_drain_patch():
    """walrus in this container allows only one sem wait on a Drain; split the
    TileContext exit drain into a chain of single-wait drains."""
    import concourse.tile as tile
    import bass_rust
    from concourse.vector_clock import ScopedClock

    if getattr(tile.TileContext, "_drain_patch", False):
        return

    def _patched(self, tick_clock, wait_clock):
        nc = self.nc
        drain_inst = nc.sync.drain()
        wait_clock.add_sem_waits(
            drain_inst.ins, ScopedClock({None: tick_clock.global_clock})
        )
        si = drain_inst.ins.sync_info
        waits = list(si.on_wait) if si is not None else []
        if len(waits) > 1:
            si.on_wait = waits[:1]
            drain_inst.ins.sync_info = si
            for w in waits[1:]:
                d2 = nc.sync.drain()
                d2.ins.sync_info = bass_rust.SyncInfo(on_wait=[w], on_update=[])
        nc.all_engine_barrier()
        assert self.sems is not None
        popped = nc._tile_sem_poison_stack.pop()
        assert popped is self._sem_poison
        nc.clear_and_free_semaphores(list(self.sems.allocated().values()))
        nc.all_engine_barrier()

    tile.TileContext._drain_and_barrier = _patched
    tile.TileContext._drain_patch = True


def _install_wait_split_patch():
    """walrus here supports only one sync-wait per instruction on several
    templates. Split any multi-wait instruction at the BIR-JSON level into a
    chain of single-wait Drain instructions on the same engine, inserted
    immediately before it."""
    import json
    import concourse.bass_utils as bu
    import concourse.bass2jax as b2j

    if getattr(bu, "_wait_split_patch", False):
        return
    orig = bu.compile_bir_kernel

    def patched(bir_json, tmpdir, neff_name="file.neff"):
        d = json.loads(bir_json)
        uid = [0]
        ndrop = [0]
        for fn in d.get("functions", []):
            for bb in fn.get("blocks", []):
                new_insts = []
                # Drop PE Ldweights that reload the exact weights already in
                # the array (same AP as the previous Ldweights in this block's
                # PE stream, no syncs attached, no weight-path use between).
                last_ldw = [None]
                for ins in bb.get("instructions", []):
                    si = ins.get("sync_info") or {}
                    waits = si.get("on_wait") or []
                    if ins.get("engine") == "PE":
                        op = ins.get("opcode")
                        if op == "Ldweights":
                            key = json.dumps(
                                [ins.get("ins"), ins.get("tile_position"),
                                 ins.get("tile_size"), ins.get("perf_mode"),
                                 ins.get("is_transpose")],
                                sort_keys=True)
                            if (key == last_ldw[0] and not waits
                                    and not (si.get("on_update") or [])):
                                ndrop[0] += 1
                                continue
                            last_ldw[0] = key
                        elif op == "Matmult":
                            if ins.get("ldweights"):
                                last_ldw[0] = None
                        elif op not in ("Drain", "EventSemaphore"):
                            last_ldw[0] = None
                    if len(waits) > 1:
                        for w in waits[:-1]:
                            uid[0] += 1
                            new_insts.append({
                                "debug": ins.get("debug", 0),
                                "engine": ins["engine"],
                                "ins": [],
                                "outs": [],
                                "is_reset_sema": False,
                                "name": f"WSPLIT-{uid[0]}",
                                "opcode": "Drain",
                                "sync_info": {"on_update": [],
                                              "on_wait": [w]},
                            })
                        si["on_wait"] = [waits[-1]]
                        ins["sync_info"] = si
                    new_insts.append(ins)
                bb["instructions"] = new_insts
        if ndrop[0]:
            print(f"[kernel2] deduped {ndrop[0]} redundant Ldweights")
        return orig(json.dumps(d).encode(), tmpdir, neff_name=neff_name)

    bu.compile_bir_kernel = patched
    b2j.compile_bir_kernel = patched
    bu._wait_split_patch = True


def _build_nc(n_octets=NOCT, stage=99, loop_reps=None):
    import concourse.bass as bass
    import concourse.mybir as mybir
    import concourse.tile as tile

    _install_drain_patch()
    f32 = mybir.dt.float32
    bf16 = mybir.dt.bfloat16
    AF = mybir.ActivationFunctionType
    OP = mybir.AluOpType

    nc = bass.Bass("TRN2")

    x_d = nc.dram_tensor("x", [BPC, T, E], bf16, kind="ExternalInput")
    wq_d = nc.dram_tensor("wq", [E, E], bf16, kind="ExternalInput")
    wk_d = nc.dram_tensor("wk", [E, E], bf16, kind="ExternalInput")
    wv_d = nc.dram_tensor("wv", [E, E], bf16, kind="ExternalInput")
    wp_d = nc.dram_tensor("wp", [E, E], bf16, kind="ExternalInput")
    w1_d = nc.dram_tensor("w1", [E, F], bf16, kind="ExternalInput")
    w2_d = nc.dram_tensor("w2", [F, E], bf16, kind="ExternalInput")
    cq_d = nc.dram_tensor("cq", [E], f32, kind="ExternalInput")
    ck_d = nc.dram_tensor("ck", [E], f32, kind="ExternalInput")
    b1_d = nc.dram_tensor("b1p", [F], f32, kind="ExternalInput")
    bp_d = nc.dram_tensor("bpb", [1, E], bf16, kind="ExternalInput")
    b2_d = nc.dram_tensor("b2b", [1, E], bf16, kind="ExternalInput")
    m0_d = nc.dram_tensor("m0", [128, NH, 128], bf16, kind="ExternalInput")
    m1_d = nc.dram_tensor("m1", [72, NH, 72], bf16, kind="ExternalInput")
    oc_d = nc.dram_tensor("onc", [128, 1], bf16, kind="ExternalInput")
    or_d = nc.dram_tensor("onr", [1, 128], bf16, kind="ExternalInput")
    it0_d = nc.dram_tensor("ind0", [1, 128], bf16, kind="ExternalInput")
    it1_d = nc.dram_tensor("ind1", [1, 128], bf16, kind="ExternalInput")
    y_d = nc.dram_tensor("y", [BPC, T, E], f32, kind="ExternalOutput")

    x_flat = x_d[:].rearrange("b t d -> (b t) d")
    y_flat = y_d[:].rearrange("b t d -> (b t) d")

    ld_eng = nc.gpsimd if USE_SWDGE else nc.sync
    mask_eng = nc.gpsimd if MASK_ENGINE == "gpsimd" else nc.vector
    vcp_eng = nc.vector if V_COPY_ENGINE == "vector" else nc.scalar

    from contextlib import ExitStack

    with tile.TileContext(nc) as tc, ExitStack() as es:
        cpool = es.enter_context(tc.tile_pool(name="const", bufs=1))
        upool = es.enter_context(tc.tile_pool(name="uc", bufs=2))
        ypool = es.enter_context(tc.tile_pool(name="yc", bufs=2))
        xpool = es.enter_context(tc.tile_pool(name="xres", bufs=2))
        hpool = es.enter_context(tc.tile_pool(name="hb", bufs=2))
        htpool = es.enter_context(tc.tile_pool(name="htb", bufs=2))
        stpool = es.enter_context(tc.tile_pool(name="st", bufs=2))
        spool = es.enter_context(tc.tile_pool(name="big", bufs=1))
        epool = es.enter_context(tc.tile_pool(name="exp", bufs=2))
        opool = (es.enter_context(tc.tile_pool(name="out", bufs=3))
                 if stage < 99 else None)
        ppool = es.enter_context(tc.tile_pool(name="ps", bufs=3, space="PSUM"))
        papool = es.enter_context(tc.tile_pool(name="pa", bufs=2, space="PSUM"))
        p1pool = es.enter_context(tc.tile_pool(name="ps1", bufs=1, space="PSUM"))

        # ---- constants ----
        wq_s = cpool.tile([128, 3, E], bf16, tag="wq")
        wk_s = cpool.tile([128, 3, E], bf16, tag="wk")
        wv_s = cpool.tile([128, 3, E], bf16, tag="wv")
        wp_s = cpool.tile([128, 3, E], bf16, tag="wp")
        w1_s = cpool.tile([128, 3, F], bf16, tag="w1")
        w2_s = cpool.tile([128, 12, E], bf16, tag="w2")
        for dst, src in ((wq_s, wq_d), (wk_s, wk_d), (wv_s, wv_d), (wp_s, wp_d),
                         (w1_s, w1_d), (w2_s, w2_d)):
            nc.sync.dma_start(dst[:], src[:].rearrange("(ko p) m -> p ko m", p=128))
        cq_s = cpool.tile([128, 3], f32, tag="cq")
        ck_s = cpool.tile([128, 3], f32, tag="ck")
        b1_s = cpool.tile([128, 12], f32, tag="b1")
        nc.sync.dma_start(cq_s[:], cq_d[:].rearrange("(mo p) -> p mo", p=128))
        nc.sync.dma_start(ck_s[:], ck_d[:].rearrange("(mo p) -> p mo", p=128))
        nc.sync.dma_start(b1_s[:], b1_d[:].rearrange("(mo p) -> p mo", p=128))
        bp_s = cpool.tile([1, E], bf16, tag="bp")
        b2_s = cpool.tile([1, E], bf16, tag="b2")
        nc.sync.dma_start(bp_s[:], bp_d[:])
        nc.sync.dma_start(b2_s[:], b2_d[:])
        m0_s = cpool.tile([128, NH, 128], bf16, tag="m0")
        m1_s = cpool.tile([72, NH, 72], bf16, tag="m1")
        oc_s = cpool.tile([128, 1], bf16, tag="onc")
        or_s = cpool.tile([1, 128], bf16, tag="onr")
        it0_s = cpool.tile([1, 128], bf16, tag="ind0")
        it1_s = cpool.tile([1, 128], bf16, tag="ind1")
        eps_s = cpool.tile([128, 1], f32, tag="eps")
        nc.vector.memset(eps_s[:], 1e-5)
        id_d = nc.dram_tensor("idm", [128, 128], bf16, kind="ExternalInput")
        id_s = cpool.tile([128, 128], bf16, tag="idm")
        nc.sync.dma_start(id_s[:], id_d[:])
        nc.sync.dma_start(m0_s[:], m0_d[:])
        nc.sync.dma_start(m1_s[:], m1_d[:])
        nc.sync.dma_start(oc_s[:], oc_d[:])
        nc.sync.dma_start(or_s[:], or_d[:])
        nc.sync.dma_start(it0_s[:], it0_d[:])
        nc.sync.dma_start(it1_s[:], it1_d[:])

        # transpose-ring round-robin between the two HWDGE rings (SP / ACT)
        tr_state = [0]

        def transpose_tile(hT, h_tile, i, w):
            eng = nc.sync if (TR_RINGS == 1 or tr_state[0] % 2 == 0) else nc.scalar
            tr_state[0] += 1
            if TRANSPOSE_3D:
                eng.dma_start_transpose(
                    hT[:, 0:3, 128 * i : 128 * i + w], h_tile[:w, i, :]
                )
            else:
                for k in range(3):
                    eng.dma_start_transpose(
                        hT[:, k, 128 * i : 128 * i + w],
                        h_tile[:w, i, 128 * k : 128 * (k + 1)],
                    )

        def load_x(o):
            x_t = xpool.tile([128, NT, E], bf16, tag="x", name=f"x{o}")
            r0 = o * TOK
            ld_eng.dma_start(
                x_t[:, 0:12, :],
                x_flat[r0 : r0 + 1536].rearrange("(g p) d -> p g d", p=128),
            )
            ld_eng.dma_start(x_t[0:64, 12, :], x_flat[r0 + 1536 : r0 + 1600])
            return x_t

        def store_chunk(y_c, o, c):
            # store the 4 (or 1) token tiles of ffn chunk c
            r0 = o * TOK + 512 * c
            if c < 3:
                ld_eng.dma_start(
                    y_flat[r0 : r0 + 512].rearrange("(g p) d -> p g d", p=128),
                    y_c[:, 0:4, :],
                )
            else:
                ld_eng.dma_start(y_flat[r0 : r0 + 64], y_c[0:64, 0, :])

        def ln_stats_tile(stats, x_t, i):
            w = TW[i]
            nc.vector.bn_stats(stats[:w, i, :], x_t[:w, i, :])

        def ln_alloc(name):
            mv = stpool.tile([128, NT, 2], f32, tag="mv", name=f"mv{name}")
            nc.vector.memset(mv[:], 1.0)
            sd = stpool.tile([128, NT], f32, tag="sd", name=f"sd{name}")
            av = stpool.tile([128, NT], f32, tag="av", name=f"av{name}")
            b0 = stpool.tile([128, NT], f32, tag="b0", name=f"b0{name}")
            h_t = hpool.tile([128, NT, E], bf16, tag="h", name=f"h{name}")
            hT = htpool.tile([128, 3, TOK], bf16, tag="hT", name=f"hT{name}")
            return (mv, sd, av, b0, h_t, hT)

        def ln_group(ln, stats, x_t, lo, hi):
            """sd/av/b0 + normalize + transpose for tiles [lo, hi). bn_aggr
            for these tiles must already have run."""
            mv, sd, av, b0, h_t, hT = ln
            nc.scalar.activation(
                sd[:, lo:hi], mv[:, lo:hi, 1], AF.Sqrt, bias=eps_s[:, 0:1]
            )
            nc.vector.reciprocal(av[:, lo:hi], sd[:, lo:hi])
            nc.vector.tensor_tensor(
                b0[:, lo:hi], mv[:, lo:hi, 0], av[:, lo:hi], OP.mult
            )
            nc.vector.tensor_scalar(b0[:, lo:hi], b0[:, lo:hi], -1.0, None, OP.mult)
            for i in range(lo, hi):
                w = TW[i]
                nc.vector.tensor_scalar(
                    h_t[:w, i, :], x_t[:w, i, :],
                    av[:w, i : i + 1], b0[:w, i : i + 1], OP.mult, OP.add,
                )
                transpose_tile(hT, h_t, i, w)

        def ln_finish(stats, x_t, name):
            """aggr + normalize into a new h tile + transposes into a new hT."""
            ln = ln_alloc(name)
            mv = ln[0]
            for i in range(NT):
                w = TW[i]
                nc.vector.bn_aggr(mv[:w, i, :], stats[:w, i, :])
            ln_group(ln, stats, x_t, 0, NT)
            return ln[5]

        def qkv(hT, name):
            """qT/kT feature-major + v token-major for one octet. Chunk-pair
            inner loops keep the stationary weights loaded across 2 matmuls."""
            qT = spool.tile([128, 3, TOK], bf16, tag="qT", name=f"qT{name}")
            kT = spool.tile([128, 3, TOK], bf16, tag="kT", name=f"kT{name}")
            for dstT, w_s, c_s in ((qT, wq_s, cq_s), (kT, wk_s, ck_s)):
                for m in range(3):
                    for cp in (0, 2):
                        pq = {}
                        for c in (cp, cp + 1):
                            pq[c] = ppool.tile([128, 512], f32, tag="pb", name="pq")
                        for k in range(3):
                            for c in (cp, cp + 1):
                                nc.tensor.matmul(
                                    pq[c][:, 0:CH],
                                    w_s[:, k, 128 * m : 128 * (m + 1)],
                                    hT[:, k, CH * c : CH * (c + 1)],
                                    start=(k == 0), stop=(k == 2),
                                )
                        for c in (cp, cp + 1):
                            # split PSUM->SBUF copies across Act/DVE so the
                            # Act queue drains before the attention exps
                            if c % 2 == 0:
                                nc.scalar.activation(
                                    dstT[:, m, CH * c : CH * (c + 1)],
                                    pq[c][:, 0:CH],
                                    AF.Identity, bias=c_s[:, m : m + 1],
                                )
                            else:
                                nc.vector.tensor_scalar(
                                    dstT[:, m, CH * c : CH * (c + 1)],
                                    pq[c][:, 0:CH],
                                    c_s[:, m : m + 1], None, OP.add,
                                )
            v_all = spool.tile([128, G, 2, E], bf16, tag="v", name=f"v{name}")
            for b in range(G):
                for tt in range(2):
                    w = 128 if tt == 0 else 72
                    col = 200 * b + 128 * tt
                    pv = ppool.tile([128, 512], f32, tag="pb")
                    for k in range(3):
                        nc.tensor.matmul(
                            pv[:w, 0:E],
                            hT[:, k, col : col + w],
                            wv_s[:, k, :],
                            start=(k == 0), stop=(k == 2),
                        )
                    vcp_eng.tensor_copy(v_all[:w, b, tt, :], pv[:w, 0:E])
            # bulk-stage the odd heads' rows down to partition base 0 (matmul
            # operands at base_partition 64 crash this walrus at runtime)
            qstg = spool.tile([64, 3, TOK], bf16, tag="qstg", name=f"qs{name}")
            kstg = spool.tile([64, 3, TOK], bf16, tag="kstg", name=f"ks{name}")
            nc.sync.dma_start(qstg[:], qT[64:128, :, :])
            nc.sync.dma_start(kstg[:], kT[64:128, :, :])
            return qT, kT, v_all, qstg, kstg

        def dump(tile_ap, nrows, row0, ncols=E):
            d = opool.tile([128, E], f32, tag="ot")
            nc.vector.tensor_copy(d[:nrows, :ncols], tile_ap)
            nc.sync.dma_start(y_flat[row0 : row0 + nrows], d[:nrows, :])

        # ================= main body =================
        loop_cm = None
        if loop_reps is not None:
            loop_cm = tc.For_i(0, loop_reps, 1)
            loop_cm.__enter__()

        # prologue (inside the loop body so each rep is self-contained)
        x_t = {}
        x_t[0] = load_x(0)
        stats0 = stpool.tile([128, NT, 6], f32, tag="stats", name="statsP")
        for i in range(NT):
            ln_stats_tile(stats0, x_t[0], i)
        hT0 = ln_finish(stats0, x_t[0], "P")
        qkv_cur = qkv(hT0, "P")

        for o in range(n_octets):
            nxt = (o + 1) % NOCT
            prep_nxt = o < n_octets - 1
            r0 = o * TOK
            qT, kT, v_all, qstg_o, kstg_o = qkv_cur

            # ---- load x(nxt) early ----
            if prep_nxt:
                x_t[nxt] = load_x(nxt)

            if stage <= 2:
                # h/hT debug: recompute dump from qT
                for i in range(4):
                    dump(qT[:, 0, 384 * i : 384 * (i + 1)], 128, r0 + 128 * i)
                qkv_cur = qkv_cur  # keep
                continue

            # ---- attention (PE runs scores one batch ahead of softmax),
            # with proj tiles interleaved as their attT columns complete ----
            attT = spool.tile([128, 3, TOK], bf16, tag="attT", name=f"attT{o}")
            stats2 = stpool.tile([128, NT, 6], f32, tag="stats", name=f"st2_{o}")
            ln2 = ln_alloc(f"B{o}")
            GRPS = [4, 8, NT]
            ln_state = [0, 0]  # g_lo, proj_done

            def proj_tile(i):
                w = TW[i]
                pp = ppool.tile([128, 512], f32, tag="pb", name="pp")
                for k in range(3):
                    nc.tensor.matmul(
                        pp[:w, 0:E],
                        attT[:, k, 128 * i : 128 * i + w],
                        wp_s[:, k, :],
                        start=(k == 0), stop=False,
                    )
                nc.tensor.matmul(
                    pp[:w, 0:E], or_s[0:1, 0:w], bp_s[:],
                    start=False, stop=False,
                )
                nc.tensor.matmul(
                    pp[:w, 0:E], id_s[:w, :w], x_t[o][:w, i, :],
                    start=False, stop=True,
                )
                nc.scalar.activation(x_t[o][:w, i, :], pp[:w, 0:E], AF.Copy)
                ln_stats_tile(stats2, x_t[o], i)
                nc.vector.bn_aggr(ln2[0][:w, i, :], stats2[:w, i, :])
                if i + 1 in GRPS:
                    ln_group(ln2, stats2, x_t[o], ln_state[0], i + 1)
                    ln_state[0] = i + 1

            pend = None
            for b in range(G + 1):
                cur = None
                if b < G:
                    c0 = 200 * b
                    expT0 = epool.tile([128, NH, T], bf16, tag="expT0")
                    expT1 = epool.tile([72, NH, 72], bf16, tag="expT1")
                    rcp = epool.tile([1, NH, T], bf16, tag="rcp")
                    def kslice(j, r, lo, hi):
                        if r == 0 or not USE_STAGE:
                            return kT[64 * r : 64 * r + 64, j, c0 + lo : c0 + hi]
                        return kstg_o[:, j, c0 + lo : c0 + hi]

                    def qslice(j, r, lo, hi):
                        if r == 0 or not USE_STAGE:
                            return qT[64 * r : 64 * r + 64, j, c0 + lo : c0 + hi]
                        return qstg_o[:, j, c0 + lo : c0 + hi]

                    # scores part1: keys 0-127, all queries
                    ps_list = []
                    for j in range(3):
                        ps = ppool.tile([128, 2, 256], f32, tag="pb", name="ps")
                        ps = ps[:, :, 0:T]
                        ps_list.append(ps)
                        for r in range(2):
                            nc.tensor.matmul(
                                ps[:, r, :],
                                kslice(j, r, 0, 128),
                                qslice(j, r, 0, T),
                                start=True, stop=True,
                            )
                    # scores part2: keys 128-200, queries 128-200
                    ps1 = p1pool.tile([72, NH, 72], f32, tag="ps1")
                    for j in range(3):
                        for r in range(2):
                            h = 2 * j + r
                            nc.tensor.matmul(
                                ps1[:, h, :],
                                kslice(j, r, 128, 200),
                                qslice(j, r, 128, 200),
                                start=True, stop=True,
                            )
                    for j in range(3):
                        nc.scalar.activation(
                            expT0[:, 2 * j : 2 * j + 2, :], ps_list[j][:], AF.Exp
                        )
                    nc.scalar.activation(expT1[:], ps1[:], AF.Exp)
                    for j in range(3):
                        mask_eng.tensor_tensor(
                            expT0[:, 2 * j : 2 * j + 2, 0:128],
                            expT0[:, 2 * j : 2 * j + 2, 0:128],
                            m0_s[:, 2 * j : 2 * j + 2, :], OP.mult,
                        )
                    # expT1's mask on DVE: it gates every denominator tail
                    # matmul, so don't queue it behind the gpsimd masks
                    nc.vector.tensor_tensor(expT1[:], expT1[:], m1_s[:],
                                            OP.mult)
                    cur = (expT0, expT1, rcp, c0)

                if pend is not None:
                    expT0, expT1, rcp, c0 = pend
                    bb = b - 1
                    sm_list = []
                    for j in range(3):
                        sm = ppool.tile([1, 512], f32, tag="pb", name="sm")
                        sm = sm[:, 0 : 2 * T].rearrange("p (h t) -> p h t", h=2)
                        sm_list.append(sm)
                        nc.tensor.matmul(
                            sm[:], oc_s[:, :], expT0[:, 2 * j : 2 * j + 2, :],
                            start=True, stop=False,
                        )
                        for r in range(2):
                            nc.tensor.matmul(
                                sm[:, r, 128:200], oc_s[0:72, :],
                                expT1[:, 2 * j + r, :],
                                start=False, stop=(r == 1),
                            )
                    with nc.allow_low_precision(reason="softmax recip in bf16"):
                        for j in range(3):
                            nc.vector.reciprocal(
                                rcp[0:1, 2 * j : 2 * j + 2, :], sm_list[j][:]
                            )

                    # attV (unnormalized) into pa banks
                    pa = papool.tile([128, 3, 256], f32, tag="pab")
                    for j in range(3):
                        for r in range(2):
                            h = 2 * j + r
                            nc.tensor.matmul(
                                pa[64 * r : 64 * r + 64, j, 0:T],
                                v_all[0:128, bb, 0, 64 * h : 64 * h + 64],
                                expT0[:, h, :],
                                start=True, stop=False,
                            )
                            nc.tensor.matmul(
                                pa[64 * r : 64 * r + 64, j, 128:200],
                                v_all[0:72, bb, 1, 64 * h : 64 * h + 64],
                                expT1[:, h, :],
                                start=False, stop=True,
                            )

                    # broadcast recip across partitions (K=1 bf16 matmuls)
                    rb = papool.tile([128, 3, 256], f32, tag="pab")
                    for j in range(3):
                        nc.tensor.matmul(
                            rb[:, j, 0:T], it0_s[:], rcp[0:1, 2 * j, :],
                            start=True, stop=False,
                        )
                        nc.tensor.matmul(
                            rb[:, j, 0:T], it1_s[:], rcp[0:1, 2 * j + 1, :],
                            start=False, stop=True,
                        )
                    rbs = epool.tile([128, 3, T], bf16, tag="rbs")
                    nc.scalar.activation(rbs[:], rb[:, :, 0:T], AF.Copy)
                    # normalize: attT[:, :, c0:c0+T] = pa * rbs  (one DVE op)
                    nc.vector.tensor_tensor(
                        attT[:, 0:3, c0 : c0 + T], pa[:, :, 0:T], rbs[:],
                        OP.mult,
                    )
                pend = cur

            if stage <= 5:
                for i in range(4):
                    dump(attT[:, 0, 384 * i : 384 * (i + 1)], 128, r0 + 128 * i)
                qkv_cur = qkv(ln_finish(stats0, x_t[nxt], f"S{o}"), f"S{o}")
                continue

            # ---- proj tail: tiles not yet emitted in the attention loop ----
            for i in range(ln_state[1], NT):
                proj_tile(i)
            h2T = ln2[5]

            if stage <= 6:
                for i in range(NT):
                    w = TW[i]
                    dump(x_t[o][:w, i, :], w, r0 + 128 * i)
                qkv_cur = qkv(ln_finish(stats0, x_t[nxt], f"S{o}"), f"S{o}")
                continue

            # ---- fused FFN: 512-token chunks, FFN1 -> FFN2 -> store ----
            # chunk token ranges: [0,512) [512,1024) [1024,1536) [1536,1600)
            ffn1_idx = [0]
            CWS = [512, 512, 512, 64]

            def ffn1_chunks(cs):
                """FFN1 for a group of chunks with shared stationary w1."""
                ucs = {}
                for c in cs:
                    ucs[c] = upool.tile([128, 12, 512], bf16, tag="uc", name="uc")
                for m in range(12):
                    pu = {}
                    for c in cs:
                        pu[c] = ppool.tile([128, 512], f32, tag="pb", name="pu")
                    for k in range(3):
                        for c in cs:
                            nc.tensor.matmul(
                                pu[c][:, 0 : CWS[c]],
                                w1_s[:, k, 128 * m : 128 * (m + 1)],
                                h2T[:, k, 512 * c : 512 * c + CWS[c]],
                                start=(k == 0), stop=(k == 2),
                            )
                    for c in cs:
                        ffn1_idx[0] += 1
                        if FFN1_DVE_EVERY and ffn1_idx[0] % FFN1_DVE_EVERY == 0:
                            nc.vector.tensor_scalar(
                                ucs[c][:, m, 0 : CWS[c]], pu[c][:, 0 : CWS[c]],
                                b1_s[:, m : m + 1], 0.0, OP.add, OP.max,
                            )
                        else:
                            nc.scalar.activation(
                                ucs[c][:, m, 0 : CWS[c]], pu[c][:, 0 : CWS[c]],
                                AF.Relu, bias=b1_s[:, m : m + 1],
                            )
                return ucs

            def ffn2_chunk(uc, c):
                ntile = 4 if c < 3 else 1
                y_c = ypool.tile([128, 4, E], f32, tag="yc")
                for t in range(ntile):
                    i = 4 * c + t
                    w = TW[i]
                    pf = ppool.tile([128, 512], f32, tag="pb", name="pf")
                    for k in range(12):
                        nc.tensor.matmul(
                            pf[:w, 0:E],
                            uc[:, k, 128 * t : 128 * t + w],
                            w2_s[:, k, :],
                            start=(k == 0), stop=False,
                        )
                    nc.tensor.matmul(
                        pf[:w, 0:E], or_s[0:1, 0:w], b2_s[:],
                        start=False, stop=False,
                    )
                    nc.tensor.matmul(
                        pf[:w, 0:E], id_s[:w, :w], x_t[o][:w, i, :],
                        start=False, stop=True,
                    )
                    nc.scalar.activation(y_c[:w, t, :], pf[:w, 0:E], AF.Copy)
                store_chunk(y_c, o, c)

            ucs = ffn1_chunks([0])
            ffn2_chunk(ucs[0], 0)
            ucs2 = ffn1_chunks([1, 2])
            ffn2_chunk(ucs2[1], 1)

            # ---- LN1(nxt): stats + finish + transposes ----
            hT_nxt = None
            if prep_nxt:
                stats1 = stpool.tile([128, NT, 6], f32, tag="stats",
                                     name=f"st1_{o}")
                for i in range(NT):
                    ln_stats_tile(stats1, x_t[nxt], i)
                hT_nxt = ln_finish(stats1, x_t[nxt], f"N{o}")

            ffn2_chunk(ucs2[2], 2)
            ucs3 = ffn1_chunks([3])
            ffn2_chunk(ucs3[3], 3)

            # ---- QKV(nxt) ----
            if prep_nxt:
                qkv_cur = qkv(hT_nxt, f"Q{o}")

        if loop_cm is not None:
            loop_cm.__exit__(None, None, None)

    return nc


def _prep_inputs(inputs):
    """Host-side folding of LN gains/biases into weights. Exact in fp32."""
    bf = ml_dtypes.bfloat16
    x = np.asarray(inputs["x"], np.float32)
    Wq = np.asarray(inputs["Wq"], np.float32)
    Wk = np.asarray(inputs["Wk"], np.float32)
    Wv = np.asarray(inputs["Wv"], np.float32)
    Wp = np.asarray(inputs["Wproj"], np.float32)
    bproj = np.asarray(inputs["bproj"], np.float32)
    W1 = np.asarray(inputs["W1"], np.float32)
    b1 = np.asarray(inputs["b1"], np.float32)
    W2 = np.asarray(inputs["W2"], np.float32)
    b2 = np.asarray(inputs["b2"], np.float32)
    g1 = np.asarray(inputs["g1"], np.float32)
    be1 = np.asarray(inputs["be1"], np.float32)
    g2 = np.asarray(inputs["g2"], np.float32)
    be2 = np.asarray(inputs["be2"], np.float32)

    s = E ** -0.5
    wq_f = (g1[:, None] * Wq) * s
    wk_f = g1[:, None] * Wk
    wv_f = g1[:, None] * Wv
    cq = (be1 @ Wq) * s
    ck = be1 @ Wk
    cv = be1 @ Wv
    bp_f = bproj + cv @ Wp
    w1_f = g2[:, None] * W1
    b1_f = b1 + be2 @ W1

    m0 = np.zeros((128, NH, 128), np.float32)
    sidx = np.arange(128)[:, None]
    tidx = np.arange(128)[None, :]
    m0[:, :, :] = (tidx >= sidx)[:, None, :]
    m1 = np.zeros((72, NH, 72), np.float32)
    si = np.arange(72)[:, None]
    ti = np.arange(72)[None, :]
    m1[:, :, :] = (ti >= si)[:, None, :]

    ind0 = np.zeros((1, 128), np.float32); ind0[0, 0:64] = 1.0
    ind1 = np.zeros((1, 128), np.float32); ind1[0, 64:128] = 1.0

    common = {
        "wq": wq_f.astype(bf), "wk": wk_f.astype(bf), "wv": wv_f.astype(bf),
        "wp": Wp.astype(bf), "w1": w1_f.astype(bf), "w2": W2.astype(bf),
        "cq": cq, "ck": ck, "b1p": b1_f,
        "bpb": bp_f.astype(bf).reshape(1, E), "b2b": b2.astype(bf).reshape(1, E),
        "m0": m0.astype(bf), "m1": m1.astype(bf),
        "onc": np.ones((128, 1), bf), "onr": np.ones((1, 128), bf),
        "ind0": ind0.astype(bf), "ind1": ind1.astype(bf),
        "idm": np.eye(128, dtype=np.float32).astype(bf),
    }
    return x.astype(bf), common


def kernel(**inputs):
    from concourse.bass_utils import run_bass_kernel_spmd

    _install_wait_split_patch()

    x, common = _prep_inputs(inputs)
    if "nc" not in _CACHE:
        _CACHE["nc"] = _build_nc()
    nc = _CACHE["nc"]
    in_maps = []
    for c in range(NCORES):
        m = dict(common)
        m["x"] = np.ascontiguousarray(x[c * BPC : (c + 1) * BPC])
        in_maps.append(m)
    res = run_bass_kernel_spmd(nc, in_maps, core_ids=list(range(NCORES)))
    out = np.concatenate([res.results[c]["y"] for c in range(NCORES)], axis=0)
    return out.astype(np.float32)
